# revision 1
# baseline (speedup 1.0000x reference)
"""Trainium2 Bass kernel: single-head causal attention, data-parallel over 8 cores.

Problem shapes (hardcoded): x [512, 256, 384] f32, Wq/Wk/Wv [384, 64] f32.
Output: [512, 256, 64] f32 = softmax(causal(q @ k^T / 8)) @ v per batch.

Sharding: pure data parallel on batch: each of 8 cores gets 64 batches;
weights replicated. No collectives.

Per-core dataflow (all on-chip compute in fp16 with fp32 PSUM accumulation):
  - SWDGE cast-DMA loads x (fp32 HBM -> fp16 SBUF), natural [t, c] layout;
    x is fully buffered in SBUF so x-load DMAs carry no slot-reuse waits
    (the 64B instruction encodings only fit 1-2 sync waits; walrus rejects
    more, and multi-dim APs shrink the budget further - tiles are kept 2D).
  - TensorE transpose mode produces xT [c, t] (6x 128x128 per batch).
  - Projections: qT,kT = W.T @ xT (weights stationary, N=256);
    v = xT.T @ W in natural [t, h] layout (xT stationary, N=64).
  - Scores computed transposed: S'[s, t] = kT.T @ qT so softmax's
    reduction direction is handled by matmul (a ones-column appended to v
    yields the denominator); exp on ScalarE with the 1/sqrt(64) scale
    folded in; causal mask = 0/1 triangle multiply on GPSIMD.
  - out[t, h(+sum)] = P'.T @ [v | 1], then per-partition reciprocal scale.
"""

import os
from contextlib import ExitStack

import numpy as np

B, T, C, H = 512, 256, 384, 64
N_CORES = 8
B_LOCAL = B // N_CORES


def build_nc(b_local=B_LOCAL, group=4, repeat=None):
    """Build the per-core Bass program.

    repeat: if set, wrap the whole batch loop in a hardware For_i that
    re-runs it `repeat` times (used only for device-time measurement:
    wall(repeat=R) - wall(repeat=1) = (R-1) * exec_time).
    """
    import concourse.mybir as mybir
    import concourse.tile as tile
    from concourse import bacc

    F32 = mybir.dt.float32
    F16 = mybir.dt.float16
    AF = mybir.ActivationFunctionType
    ALU = mybir.AluOpType

    assert b_local % group == 0

    nc = bacc.Bacc()
    x = nc.declare_dram_parameter("x", [b_local, T, C], F32, isOutput=False)
    wq = nc.declare_dram_parameter("Wq", [C, H], F32, isOutput=False)
    wk = nc.declare_dram_parameter("Wk", [C, H], F32, isOutput=False)
    wv = nc.declare_dram_parameter("Wv", [C, H], F32, isOutput=False)
    out = nc.declare_dram_parameter("out", [b_local, T, H], F32, isOutput=True)

    NT = T // 128  # 2 token chunks
    NCC = C // 128  # 3 contraction chunks
    H1 = H + 1
    SCALE = 1.0 / np.sqrt(H)

    with tile.TileContext(nc) as tc, ExitStack() as ctx:
        const = ctx.enter_context(tc.tile_pool(name="const", bufs=1))
        xnat_p = ctx.enter_context(
            tc.tile_pool(name="xnat", bufs=b_local // group))
        xt_ps_p = ctx.enter_context(tc.tile_pool(name="xt_ps", bufs=2, space="PSUM"))
        xt_p = ctx.enter_context(tc.tile_pool(name="xt", bufs=3))
        qk_ps_p = ctx.enter_context(tc.tile_pool(name="qk_ps", bufs=2, space="PSUM"))
        qk_p = ctx.enter_context(tc.tile_pool(name="qk", bufs=2))
        v_ps_p = ctx.enter_context(tc.tile_pool(name="v_ps", bufs=2, space="PSUM"))
        v_p = ctx.enter_context(tc.tile_pool(name="v", bufs=2))
        s_ps_p = ctx.enter_context(tc.tile_pool(name="s_ps", bufs=1, space="PSUM"))
        p_p = ctx.enter_context(tc.tile_pool(name="p", bufs=2))
        o_ps_p = ctx.enter_context(tc.tile_pool(name="o_ps", bufs=1, space="PSUM"))
        o_p = ctx.enter_context(tc.tile_pool(name="o", bufs=2))
        r_p = ctx.enter_context(tc.tile_pool(name="r", bufs=2))

        # --- constants ---
        # fp16 weights, [128, chunk*H] with c = chunk*128 + partition.
        # Load fp32 via HWDGE (keeps the SWDGE sem lanes exclusive to x
        # loads), then cast on DVE.
        wq_sb = const.tile([128, NCC * H], F16, tag="wq")
        wk_sb = const.tile([128, NCC * H], F16, tag="wk")
        wv_sb = const.tile([128, NCC * H], F16, tag="wv")
        w_stage = const.tile([128, 3 * NCC * H], F32, tag="w_stage")
        for i, w in enumerate((wq, wk, wv)):
            nc.sync.dma_start(
                w_stage[:, i * NCC * H:(i + 1) * NCC * H],
                w.rearrange("(a p) h -> p a h", p=128))
        nc.vector.tensor_copy(wq_sb[:], w_stage[:, 0:NCC * H])
        nc.vector.tensor_copy(wk_sb[:], w_stage[:, NCC * H:2 * NCC * H])
        nc.vector.tensor_copy(wv_sb[:], w_stage[:, 2 * NCC * H:3 * NCC * H])

        ones = const.tile([128, 128], F16, tag="ones")
        nc.vector.memset(ones[:], 1.0)
        # tri[p, j] = 1 if j >= p else 0   (keep s <= t in S'[s, t] layout)
        tri = const.tile([128, 128], F16, tag="tri")
        nc.gpsimd.affine_select(
            tri[:], ones[:], pattern=[[1, 128]], compare_op=ALU.is_ge,
            fill=0.0, base=0, channel_multiplier=-1,
        )
        # identity for TensorE transpose
        ident = const.tile([128, 128], F16, tag="ident")
        nc.gpsimd.affine_select(
            ident[:], ones[:], pattern=[[1, 128]], compare_op=ALU.is_equal,
            fill=0.0, base=0, channel_multiplier=-1,
        )

        loop_cm = tc.For_i(0, repeat, 1) if repeat is not None else None
        if loop_cm is not None:
            loop_cm.__enter__()
        for g in range(b_local // group):
            # fp32 -> fp16 cast during DMA (SWDGE); x natural layout,
            # columns [(bb*NT + n)*C + c].
            xnat = xnat_p.tile([128, group * NT * C], F16, tag="xnat")
            nc.gpsimd.dma_start(
                xnat[:],
                x[g * group:(g + 1) * group].rearrange("b (n p) c -> p b n c", p=128),
            )
            for bb in range(group):
                b = g * group + bb
                # --- transpose x -> xT [c, t]; columns [cc*T + t] ---
                xt_ps = xt_ps_p.tile([128, NCC * T], F16, tag="xt_ps")
                for cc in range(NCC):
                    for n in range(NT):
                        nc.tensor.transpose(
                            xt_ps[:, cc * T + n * 128:cc * T + (n + 1) * 128],
                            xnat[:, (bb * NT + n) * C + cc * 128:
                                 (bb * NT + n) * C + (cc + 1) * 128],
                            ident[:],
                        )
                xt = xt_p.tile([128, NCC * T], F16, tag="xt")
                nc.vector.tensor_copy(xt[:], xt_ps[:])

                # --- projections ---
                # qT | kT side by side: [64, 0:256]=qT, [64, 256:512]=kT
                qk_ps = qk_ps_p.tile([H, 2 * T], F32, tag="qk_ps")
                v_ps = v_ps_p.tile([128, NT * H], F32, tag="v_ps")
                for cc in range(NCC):
                    st = dict(start=(cc == 0), stop=(cc == NCC - 1))
                    nc.tensor.matmul(
                        qk_ps[:, 0:T], wq_sb[:, cc * H:(cc + 1) * H],
                        xt[:, cc * T:(cc + 1) * T], **st)
                for cc in range(NCC):
                    st = dict(start=(cc == 0), stop=(cc == NCC - 1))
                    nc.tensor.matmul(
                        qk_ps[:, T:2 * T], wk_sb[:, cc * H:(cc + 1) * H],
                        xt[:, cc * T:(cc + 1) * T], **st)
                for n in range(NT):
                    for cc in range(NCC):
                        st = dict(start=(cc == 0), stop=(cc == NCC - 1))
                        nc.tensor.matmul(
                            v_ps[:, n * H:(n + 1) * H],
                            xt[:, cc * T + n * 128:cc * T + (n + 1) * 128],
                            wv_sb[:, cc * H:(cc + 1) * H], **st)

                qk = qk_p.tile([H, 2 * T], F16, tag="qk")
                nc.scalar.copy(qk[:], qk_ps[:])

                # v_ext = [v | 1]: ones column gives the softmax denominator
                vx = v_p.tile([128, NT * H1], F16, tag="vx")
                for n in range(NT):
                    nc.scalar.copy(
                        vx[:, n * H1:n * H1 + H], v_ps[:, n * H:(n + 1) * H])
                    nc.gpsimd.memset(vx[:, n * H1 + H:(n + 1) * H1], 1.0)

                # --- scores (transposed): S'[s, t] = kT.T @ qT ---
                # S0: s in [0,128), t in [0,256); S1: s in [128,256), t in [128,256)
                s_ps = s_ps_p.tile([128, T + 128], F32, tag="s_ps")
                nc.tensor.matmul(s_ps[:, 0:T], qk[:, T:T + 128], qk[:, 0:T])
                nc.tensor.matmul(s_ps[:, T:T + 128], qk[:, T + 128:2 * T],
                                 qk[:, 128:T])

                # --- exp (scale folded in); causal mask on diagonal blocks ---
                p0 = p_p.tile([128, T], F16, tag="p0")
                p1 = p_p.tile([128, 128], F16, tag="p1")
                nc.scalar.activation(p0[:], s_ps[:, 0:T], AF.Exp, scale=SCALE)
                nc.scalar.activation(p1[:], s_ps[:, T:T + 128], AF.Exp, scale=SCALE)
                nc.gpsimd.tensor_mul(p0[:, 0:128], p0[:, 0:128], tri[:])
                nc.gpsimd.tensor_mul(p1[:], p1[:], tri[:])

                # --- out[t, h(+denominator)] = P'.T @ v_ext ---
                o_ps = o_ps_p.tile([128, NT * H1], F32, tag="o_ps")
                nc.tensor.matmul(o_ps[:, 0:H1], p0[:, 0:128], vx[:, 0:H1])
                nc.tensor.matmul(o_ps[:, H1:2 * H1], p0[:, 128:T], vx[:, 0:H1],
                                 start=True, stop=False)
                nc.tensor.matmul(o_ps[:, H1:2 * H1], p1[:], vx[:, H1:2 * H1],
                                 start=False, stop=True)

                # --- normalize: out / denominator, write fp32 ---
                rec = r_p.tile([128, NT], F32, tag="rec")
                nc.vector.reciprocal(rec[:], o_ps[:, H::H1])
                ob = o_p.tile([128, NT * H], F32, tag="ob")
                for n in range(NT):
                    nc.vector.tensor_scalar_mul(
                        ob[:, n * H:(n + 1) * H],
                        o_ps[:, n * H1:n * H1 + H],
                        rec[:, n:n + 1])
                    nc.sync.dma_start(
                        out[b, n * 128:(n + 1) * 128, :],
                        ob[:, n * H:(n + 1) * H])
        if loop_cm is not None:
            loop_cm.__exit__(None, None, None)

    nc.compile()
    return nc


_CACHED = {}


def _make_runner(nc):
    """Build a cached shard_map'd jit for an SPMD Bass program.

    Mirrors concourse.bass2jax.run_bass_via_pjrt, but without output-buffer
    donation so the compiled executable can be re-invoked with
    device-resident arrays (no host transfers on warm calls).
    """
    import jax
    from jax.experimental.shard_map import shard_map
    from jax.sharding import Mesh, NamedSharding, PartitionSpec

    import concourse.mybir as mybir
    from concourse.bass2jax import (
        _bass_exec_p, install_neuronx_cc_hook, partition_id_tensor)

    install_neuronx_cc_hook()

    partition_name = (
        nc.partition_id_tensor.name if nc.partition_id_tensor else None)
    in_names, out_names, out_avals, zero_outs = [], [], [], []
    for alloc in nc.m.functions[0].allocations:
        if not isinstance(alloc, mybir.MemoryLocationSet):
            continue
        name = alloc.memorylocations[0].name
        if alloc.kind == "ExternalInput":
            if name != partition_name:
                in_names.append(name)
        elif alloc.kind == "ExternalOutput":
            out_names.append(name)
            shape = tuple(alloc.tensor_shape)
            dtype = mybir.dt.np(alloc.dtype)
            out_avals.append(jax.core.ShapedArray(shape, dtype))
            zero_outs.append(np.zeros(shape, dtype))
    n_params = len(in_names)
    all_in = in_names + out_names
    if partition_name is not None:
        all_in = all_in + [partition_name]

    def _body(*args):
        operands = list(args)
        if partition_name is not None:
            operands.append(partition_id_tensor())
        outs = _bass_exec_p.bind(
            *operands,
            out_avals=tuple(out_avals),
            in_names=tuple(all_in),
            out_names=tuple(out_names),
            lowering_input_output_aliases=(),
            sim_require_finite=False,
            sim_require_nnan=False,
            nc=nc,
        )
        return tuple(outs)

    devices = jax.devices()[:N_CORES]
    mesh = Mesh(np.asarray(devices), ("core",))
    spec = PartitionSpec("core")
    n_args = n_params + len(out_names)
    sharded = jax.jit(
        shard_map(
            _body, mesh=mesh, in_specs=(spec,) * n_args,
            out_specs=(spec,) * len(out_names), check_rep=False,
        ),
        keep_unused=True,
    )
    sharding = NamedSharding(mesh, spec)
    return sharded, in_names, zero_outs, sharding


def _get_runner():
    if "runner" not in _CACHED:
        _CACHED["runner"] = _make_runner(build_nc())
    return _CACHED["runner"]


def _device_inputs(x, Wq, Wk, Wv):
    """Concat per-core inputs on axis 0 (shard_map layout) and device_put."""
    import jax

    sharded, in_names, zero_outs, sharding = _get_runner()
    x = np.ascontiguousarray(x, dtype=np.float32)
    assert x.shape == (B, T, C)
    host = {
        "x": x,
        "Wq": np.concatenate([np.asarray(Wq, np.float32)] * N_CORES, axis=0),
        "Wk": np.concatenate([np.asarray(Wk, np.float32)] * N_CORES, axis=0),
        "Wv": np.concatenate([np.asarray(Wv, np.float32)] * N_CORES, axis=0),
    }
    args = [host[n] for n in in_names]
    args += [
        np.zeros((N_CORES * z.shape[0], *z.shape[1:]), z.dtype) for z in zero_outs
    ]
    return [jax.device_put(a, sharding) for a in args]


def kernel(x, Wq, Wk, Wv):
    sharded, _, _, _ = _get_runner()
    args = _device_inputs(x, Wq, Wk, Wv)
    (out,) = sharded(*args)
    return np.asarray(out)



# revision 3
# speedup vs baseline: 1.3996x; 1.3996x over previous
"""Trainium2 Bass kernel v2: single-head causal attention, data-parallel over 8 cores.

Problem shapes (hardcoded): x [512, 256, 384] f32, Wq/Wk/Wv [384, 64] f32.
Output: [512, 256, 64] f32 = softmax(causal(q @ k^T / 8)) @ v per batch.

Sharding: pure data parallel on batch: each of 8 cores gets 64 batches;
weights replicated. No collectives.

v2 redesign vs v1 (v1 sim: 127us, ACT 90% busy):
  - Wq|Wk packed into one M=128 stationary: projection qkT in 3 N=512
    matmuls per 2-batch pair (halves qk-proj PE time and copies).
  - Scores read the packed qk tile with explicit tile_position=(0,0)
    (stationary kT from partitions 64:128, moving qT from 0:64).
  - One exp per batch over contiguous [128, 384] PSUM (S0|S1), scale folded.
  - Work spread: DVE (xt copy, qk b0 copy, recip, norm-mul), ACT (exp,
    vx copy, qk b1 copy), Pool (x SWDGE, tri masks, ones memset), SP (out).
  - Outputs written f16 (halves out-DMA descriptor cost; host casts f32;
    adds ~1e-4 rel err), one DMA per 4 batches.
  - x loads in 8-batch SWDGE cast groups, fully prefetched (bufs=8).
"""

import os
from contextlib import ExitStack

import numpy as np

B, T, C, H = 512, 256, 384, 64
N_CORES = 8
B_LOCAL = B // N_CORES


def build_nc(b_local=B_LOCAL, repeat=None):
    """Build the per-core Bass program.

    repeat: if set, wrap the whole batch loop in a hardware For_i that
    re-runs it `repeat` times (used only for device-time measurement).
    """
    import concourse.mybir as mybir
    import concourse.tile as tile
    from concourse import bacc

    F32 = mybir.dt.float32
    F16 = mybir.dt.float16
    AF = mybir.ActivationFunctionType
    ALU = mybir.AluOpType

    assert b_local % 8 == 0
    n_g4 = b_local // 4

    nc = bacc.Bacc()
    x = nc.declare_dram_parameter("x", [b_local, T, C], F32, isOutput=False)
    wq = nc.declare_dram_parameter("Wq", [C, H], F32, isOutput=False)
    wk = nc.declare_dram_parameter("Wk", [C, H], F32, isOutput=False)
    wv = nc.declare_dram_parameter("Wv", [C, H], F32, isOutput=False)
    out = nc.declare_dram_parameter("out", [b_local, T, H], F16, isOutput=True)

    NT = T // 128   # 2 token chunks
    NCC = C // 128  # 3 contraction chunks
    SCALE = 1.0 / np.sqrt(H)

    with tile.TileContext(nc) as tc, ExitStack() as ctx:
        const = ctx.enter_context(tc.tile_pool(name="const", bufs=1))
        xnat_p = ctx.enter_context(tc.tile_pool(name="xnat", bufs=4))
        xt_ps_p = ctx.enter_context(tc.tile_pool(name="xt_ps", bufs=2, space="PSUM"))
        xt_p = ctx.enter_context(tc.tile_pool(name="xt", bufs=3))
        qk_ps_p = ctx.enter_context(tc.tile_pool(name="qk_ps", bufs=2, space="PSUM"))
        qk_p = ctx.enter_context(tc.tile_pool(name="qk", bufs=3))
        qks_p = ctx.enter_context(tc.tile_pool(name="qks", bufs=3))
        v_ps_p = ctx.enter_context(tc.tile_pool(name="v_ps", bufs=1, space="PSUM"))
        vx_p = ctx.enter_context(tc.tile_pool(name="vx", bufs=4))
        s_ps_p = ctx.enter_context(tc.tile_pool(name="s_ps", bufs=2, space="PSUM"))
        p_p = ctx.enter_context(tc.tile_pool(name="p", bufs=6))
        o_ps_p = ctx.enter_context(tc.tile_pool(name="o_ps", bufs=1, space="PSUM"))
        r_p = ctx.enter_context(tc.tile_pool(name="r", bufs=4))
        ob_p = ctx.enter_context(tc.tile_pool(name="ob", bufs=2))

        # --- constants ---
        # Packed projection weights: wqk[p, cc*128 + j] = Wq[cc*128+p, j]
        # for j<64 else Wk[cc*128+p, j-64].  wv[p, cc*64 + h].
        wqk_sb = const.tile([128, NCC * 128], F16, tag="wqk")
        wv_sb = const.tile([128, NCC * H], F16, tag="wv")
        w_stage = const.tile([128, 3 * NCC * H], F32, tag="w_stage")
        for i, w in enumerate((wq, wk, wv)):
            nc.sync.dma_start(
                w_stage[:, i * NCC * H:(i + 1) * NCC * H],
                w.rearrange("(a p) h -> p a h", p=128))
        wqk_v = wqk_sb[:].rearrange("p (a x) -> p a x", a=NCC)
        nc.vector.tensor_copy(
            wqk_v[:, :, 0:H],
            w_stage[:, 0:NCC * H].rearrange("p (a h) -> p a h", a=NCC))
        nc.vector.tensor_copy(
            wqk_v[:, :, H:128],
            w_stage[:, NCC * H:2 * NCC * H].rearrange("p (a h) -> p a h", a=NCC))
        nc.vector.tensor_copy(wv_sb[:], w_stage[:, 2 * NCC * H:3 * NCC * H])

        ones = const.tile([128, 128], F16, tag="ones")
        nc.vector.memset(ones[:], 1.0)
        # tri[p, j] = 1 if j >= p else 0   (keep s <= t in S'[s, t] layout)
        tri = const.tile([128, 128], F16, tag="tri")
        nc.gpsimd.affine_select(
            tri[:], ones[:], pattern=[[1, 128]], compare_op=ALU.is_ge,
            fill=0.0, base=0, channel_multiplier=-1,
        )
        # identity for TensorE transpose
        ident = const.tile([128, 128], F16, tag="ident")
        nc.gpsimd.affine_select(
            ident[:], ones[:], pattern=[[1, 128]], compare_op=ALU.is_equal,
            fill=0.0, base=0, channel_multiplier=-1,
        )

        n_it = b_local // 2
        # x-load groups: [0:2), [2:8), then 8-batch groups. group_of[i] maps
        # a 2-batch iter to (group_idx, first_batch_of_group).
        group_spans = [(0, 2), (2, 8)] + [
            (8 * k, 8 * (k + 1)) for k in range(1, b_local // 8)]
        iter_group = {}
        for gi, (lo, hi) in enumerate(group_spans):
            for i in range(lo // 2, hi // 2):
                iter_group[i] = (gi, lo)
        # issue schedule: body index -> list of group indices to load
        issue_at = {0: [0, 1]}
        for gi in range(2, len(group_spans)):
            lo = group_spans[gi][0]
            issue_at.setdefault(max(1, lo // 2 - 5), []).append(gi)

        state = {}

        def stage_a(i):
            """x loads + transposes + xt copies for iter i."""
            for gi in issue_at.get(i, []):
                lo, hi = group_spans[gi]
                t = xnat_p.tile([128, 8 * NT * C], F16, tag="xnat", name="xnat")
                nc.gpsimd.dma_start(
                    t[:, 0:(hi - lo) * NT * C],
                    x[lo:hi].rearrange("b (n p) c -> p b n c", p=128),
                )
                state[gi] = t
            gi, lo = iter_group[i]
            xnat = state[gi]
            xt2 = xt_p.tile([128, NCC * 2 * T], F16, tag="xt2")
            for bb in range(2):
                lb = 2 * i + bb - lo
                xt_ps = xt_ps_p.tile([128, NCC * T], F16, tag="xt_ps")
                for cc in range(NCC):
                    for n in range(NT):
                        nc.tensor.transpose(
                            xt_ps[:, cc * T + n * 128:cc * T + (n + 1) * 128],
                            xnat[:, (lb * NT + n) * C + cc * 128:
                                 (lb * NT + n) * C + (cc + 1) * 128],
                            ident[:],
                        )
                nc.vector.tensor_copy(
                    xt2[:].rearrange(
                        "p (cc t) -> p cc t", cc=NCC)[:, :, bb * T:(bb + 1) * T],
                    xt_ps[:].rearrange("p (cc t) -> p cc t", cc=NCC),
                )
            state[("xt2", i)] = xt2

        def stage_b(i):
            """projections, scores, exp, mask for iter i."""
            xt2 = state.pop(("xt2", i))
            # qk proj: qkT packed [q:0:64 | k:64:128], per-batch col groups
            qk_ps = qk_ps_p.tile([128, 2 * T], F32, tag="qk_ps")
            qk = qk_p.tile([128, 2 * T], F16, tag="qk")
            for bb in range(2):
                for cc in range(NCC):
                    nc.tensor.matmul(
                        qk_ps[:, bb * T:(bb + 1) * T],
                        wqk_sb[:, cc * 128:(cc + 1) * 128],
                        xt2[:].rearrange("p (cc t) -> p cc t", cc=NCC)
                        [:, cc, bb * T:(bb + 1) * T],
                        start=(cc == 0), stop=(cc == NCC - 1))
                if bb == 0:
                    nc.vector.tensor_copy(qk[:, 0:T], qk_ps[:, 0:T])
                else:
                    nc.scalar.copy(qk[:, T:2 * T], qk_ps[:, T:2 * T])

            # v projection (natural layout), both batches
            v_ps = v_ps_p.tile([128, 2 * NT * H], F32, tag="v_ps")
            for bb in range(2):
                for n in range(NT):
                    for cc in range(NCC):
                        nc.tensor.matmul(
                            v_ps[:, (bb * NT + n) * H:(bb * NT + n + 1) * H],
                            xt2[:, cc * 2 * T + bb * T + n * 128:
                                 cc * 2 * T + bb * T + (n + 1) * 128],
                            wv_sb[:, cc * H:(cc + 1) * H],
                            start=(cc == 0), stop=(cc == NCC - 1))
            # vx per batch: [v0 (64) | 1 | v1 (64) | 1] = 130 cols
            vx = vx_p.tile([128, 2 * 2 * (H + 1)], F16, tag="vx")
            vx_v = vx[:].rearrange("p (b m h) -> p b m h", b=2, m=2)
            nc.scalar.copy(
                vx_v[:, :, :, 0:H],
                v_ps[:].rearrange("p (b m h) -> p b m h", b=2, m=2))
            nc.gpsimd.memset(vx_v[:, :, :, H:H + 1], 1.0)
            state[("vx", i)] = vx

            # shift the k-half down to partitions 0:64 (walrus requires both
            # matmul operands at the same SBUF partition base)
            qks = qks_p.tile([64, 2 * T], F16, tag="qks")
            nc.sync.dma_start(qks[:], qk[64:128, :])
            state[("qk", i)] = (qk, qks)

        def stage_c(i):
            """scores, exp, tri mask for iter i."""
            qk, qks = state.pop(("qk", i))
            ps = []
            for bb in range(2):
                # scores S'[s, t] (transposed): lhsT = kT (shifted), rhs = qT
                s_ps = s_ps_p.tile([128, T + 128], F32, tag="s_ps")
                nc.tensor.matmul(
                    s_ps[:, 0:T],
                    qks[0:64, bb * T:bb * T + 128],
                    qk[0:64, bb * T:(bb + 1) * T])
                nc.tensor.matmul(
                    s_ps[:, T:T + 128],
                    qks[0:64, bb * T + 128:(bb + 1) * T],
                    qk[0:64, bb * T + 128:(bb + 1) * T])
                # exp (scale folded); tri mask on the 2 diag blocks
                p = p_p.tile([128, T + 128], F16, tag="p")
                nc.scalar.activation(p[:], s_ps[:], AF.Exp, scale=SCALE)
                nc.gpsimd.tensor_mul(
                    p[:].rearrange("p (a j) -> p a j", a=3)[:, 0::2, :],
                    p[:].rearrange("p (a j) -> p a j", a=3)[:, 0::2, :],
                    tri[:].unsqueeze(1).broadcast_to((128, 2, 128)))
                ps.append(p)
            state[("p", i)] = ps

        def stage_d(i):
            """AV, normalize, out DMA for iter i."""
            vx = state.pop(("vx", i))
            ps = state.pop(("p", i))
            if i % 2 == 0:
                state["ob"] = ob_p.tile([128, 4 * NT * H], F16, tag="ob", name="ob")
            ob = state["ob"]
            # o_ps holds both batches: [b0 (n0 h|d, n1 h|d) | b1 ...]
            o_ps = o_ps_p.tile([128, 4 * (H + 1)], F32, tag="o_ps")
            for bb in range(2):
                p = ps[bb]
                vb = bb * 2 * (H + 1)
                nc.tensor.matmul(
                    o_ps[:, vb:vb + H + 1], p[:, 0:128],
                    vx[:, vb:vb + H + 1])
                nc.tensor.matmul(
                    o_ps[:, vb + H + 1:vb + 2 * (H + 1)], p[:, 128:T],
                    vx[:, vb:vb + H + 1], start=True, stop=False)
                nc.tensor.matmul(
                    o_ps[:, vb + H + 1:vb + 2 * (H + 1)], p[:, T:T + 128],
                    vx[:, vb + H + 1:vb + 2 * (H + 1)],
                    start=False, stop=True)

            # normalize both batches: out / denominator -> ob (f16)
            o_v = o_ps[:].rearrange("p (m x) -> p m x", m=4)
            rec = r_p.tile([128, 4], F32, tag="rec")
            nc.vector.reciprocal(
                rec[:].rearrange("p (m x) -> p m x", m=4),
                o_v[:, :, H:H + 1])
            c0 = (i % 2) * 2 * NT * H
            nc.vector.tensor_mul(
                ob[:, c0:c0 + 2 * NT * H].rearrange("p (m h) -> p m h", m=4),
                o_v[:, :, 0:H],
                rec[:].rearrange(
                    "p (m x) -> p m x", m=4).broadcast_to((128, 4, H)),
            )
            if i % 2 == 1:
                g4 = i // 2
                nc.sync.dma_start(
                    out[g4 * 4:(g4 + 1) * 4].rearrange(
                        "b (n p) h -> p b n h", p=128),
                    ob[:].rearrange("p (b n h) -> p b n h", b=4, n=NT))

        loop_cm = tc.For_i(0, repeat, 1) if repeat is not None else None
        if loop_cm is not None:
            loop_cm.__enter__()
        for j in range(n_it + 3):
            if j < n_it:
                stage_a(j)
            if 1 <= j <= n_it:
                stage_b(j - 1)
            if 2 <= j <= n_it + 1:
                stage_c(j - 2)
            if j >= 3:
                stage_d(j - 3)
        if loop_cm is not None:
            loop_cm.__exit__(None, None, None)

    nc.compile()
    return nc


_CACHED = {}


def _make_runner(nc):
    """Build a cached shard_map'd jit for an SPMD Bass program."""
    import jax
    from jax.experimental.shard_map import shard_map
    from jax.sharding import Mesh, NamedSharding, PartitionSpec

    import concourse.mybir as mybir
    from concourse.bass2jax import (
        _bass_exec_p, install_neuronx_cc_hook, partition_id_tensor)

    install_neuronx_cc_hook()

    partition_name = (
        nc.partition_id_tensor.name if nc.partition_id_tensor else None)
    in_names, out_names, out_avals, zero_outs = [], [], [], []
    for alloc in nc.m.functions[0].allocations:
        if not isinstance(alloc, mybir.MemoryLocationSet):
            continue
        name = alloc.memorylocations[0].name
        if alloc.kind == "ExternalInput":
            if name != partition_name:
                in_names.append(name)
        elif alloc.kind == "ExternalOutput":
            out_names.append(name)
            shape = tuple(alloc.tensor_shape)
            dtype = mybir.dt.np(alloc.dtype)
            out_avals.append(jax.core.ShapedArray(shape, dtype))
            zero_outs.append(np.zeros(shape, dtype))
    n_params = len(in_names)
    all_in = in_names + out_names
    if partition_name is not None:
        all_in = all_in + [partition_name]

    def _body(*args):
        operands = list(args)
        if partition_name is not None:
            operands.append(partition_id_tensor())
        outs = _bass_exec_p.bind(
            *operands,
            out_avals=tuple(out_avals),
            in_names=tuple(all_in),
            out_names=tuple(out_names),
            lowering_input_output_aliases=(),
            sim_require_finite=False,
            sim_require_nnan=False,
            nc=nc,
        )
        return tuple(outs)

    devices = jax.devices()[:N_CORES]
    mesh = Mesh(np.asarray(devices), ("core",))
    spec = PartitionSpec("core")
    n_args = n_params + len(out_names)
    sharded = jax.jit(
        shard_map(
            _body, mesh=mesh, in_specs=(spec,) * n_args,
            out_specs=(spec,) * len(out_names), check_rep=False,
        ),
        keep_unused=True,
    )
    sharding = NamedSharding(mesh, spec)
    return sharded, in_names, zero_outs, sharding


def _get_runner():
    if "runner" not in _CACHED:
        _CACHED["runner"] = _make_runner(build_nc())
    return _CACHED["runner"]


def _device_inputs(x, Wq, Wk, Wv):
    """Concat per-core inputs on axis 0 (shard_map layout) and device_put."""
    import jax

    sharded, in_names, zero_outs, sharding = _get_runner()
    x = np.ascontiguousarray(x, dtype=np.float32)
    assert x.shape == (B, T, C)
    host = {
        "x": x,
        "Wq": np.concatenate([np.asarray(Wq, np.float32)] * N_CORES, axis=0),
        "Wk": np.concatenate([np.asarray(Wk, np.float32)] * N_CORES, axis=0),
        "Wv": np.concatenate([np.asarray(Wv, np.float32)] * N_CORES, axis=0),
    }
    args = [host[n] for n in in_names]
    args += [
        np.zeros((N_CORES * z.shape[0], *z.shape[1:]), z.dtype) for z in zero_outs
    ]
    return [jax.device_put(a, sharding) for a in args]


def kernel(x, Wq, Wk, Wv):
    sharded, _, _, _ = _get_runner()
    args = _device_inputs(x, Wq, Wk, Wv)
    (out,) = sharded(*args)
    return np.asarray(out).astype(np.float32)
